# revision 30
# baseline (speedup 1.0000x reference)
"""GraphSage 2-level mean-aggregator GNN on 8 trn2 NeuronCores.

Strategy (memory-bound: dif_mat_1 [6000, 48000] must stream through the chip):

  * Shard the level-1 diffusion matmul over its CONTRACTION dim (48000 src
    selections): core k owns 6000 src columns.  The dif shard streams as
    fp8_e4m3 (prescaled by S1; the 1/S1 rescale is folded into w1's top
    half host-side) -- numerically safe because agg1 is ~0.003 of the
    concatenated dst-feature magnitude.  fp8 halves both HBM traffic and,
    via DoubleRow double-pumping (2 contraction rows per PE pass), the
    TensorEngine time.
  * Each core computes the partial agg1^T = src_feat^T @ dif^T [128, 6144]
    in 12 dst blocks of 512, staged to DRAM as feature-major [dst-chunk,
    feat, 128] chunks in 2 groups of [8, 4] dst blocks.  The chunked
    layout makes the ReduceScatter flat-split land exactly on chunk
    boundaries, so each core's RS output IS its agg1^T slice -- no
    transposes anywhere on the partial path.
  * Per group: ReduceScatter sums the 8 partials; the core runs the
    level-1 dense (relu(concat@w1)) on its 512/256-row slice only.  The
    group-0 chain hides under the stream tail; group 1 is kept tiny so its
    post-stream RS is latency- not size-bound.
  * Level 0 is linear in h1, so h1 is never redistributed: each core
    multiplies its own h1 rows (DMA-transposed to natural layout) into
    every core's level-0 matrix (host-built fp8 [difT0exp*S0 | one-hot-E]
    rows for its slice, all 8 target blocks), giving a [1024, 256] stack
    of per-target partial sums.  A second ReduceScatter sums the stacks
    in-flight and hands each core its own 128 targets' c0 directly.
  * Final dense + softmax per core on its 128 target rows (1/S0 folded
    into w2's top half); host concatenates the eight [128, 40] outputs.
"""

import sys

import ml_dtypes
import numpy as np

sys.path.insert(0, "/opt/trn_rl_repo")

from concourse import bacc, bass_utils, mybir, tile

F32 = mybir.dt.float32
BF16 = mybir.dt.bfloat16
F8 = mybir.dt.float8e4

# Problem dims (hardcoded per spec)
N, F = 100000, 128
N1, D1, S1 = 60000, 6000, 48000
D0, S0 = 1024, 5000
H, C = 128, 40
NCORES = 8
CH = S1 // NCORES   # 6000 src columns per core
KT = 48             # contraction k-tiles of 128 (6144 = padded 6000)
KP = KT * 128       # 6144
NPAIR = KT // 2     # 24 DoubleRow k-pairs
GP = 8              # pairs per stream DMA
NG = NPAIR // GP    # 3 stream DMAs per dst block
JW = 512            # dst-block width for the big matmul
JB = 12             # dst blocks (6144 = padded 6000)
DP = JB * JW        # 6144
JBG = [8, 2, 2]     # j-blocks per collective group (small tail groups)
NAG = len(JBG)
GW = [n * JW for n in JBG]          # dst rows per group: 5120, 1024
OWN = [w // NCORES for w in GW]     # own rows per group: 640, 128
OWNC = [o // 128 for o in OWN]      # own 128-chunks per group: 5, 1
CTOT = sum(OWNC)                    # total own chunks: 6
D0SH = D0 // NCORES                 # 128 target rows per core

TRACE = False
TRACE_KW = {}
LAST = None

_nc = None


def _build(repeat=1):
    nc = bacc.Bacc(
        "TRN2",
        target_bir_lowering=False,
        debug=False,
        enable_asserts=False,
        num_devices=NCORES,
    )
    difT1 = nc.dram_tensor("difT1", [JB, NG, 128, GP, 2, JW], F8, kind="ExternalInput")
    sfeat = nc.dram_tensor("sfeat", [128, NPAIR, 2, F], F8, kind="ExternalInput")
    dfT2 = nc.dram_tensor("dfT2", [F, CTOT * 128], BF16, kind="ExternalInput")
    d0a = nc.dram_tensor("d0a", [128, CTOT, NCORES, 2 * H], F8, kind="ExternalInput")
    w1t = nc.dram_tensor("w1t", [2 * F, H], BF16, kind="ExternalInput")
    w2t = nc.dram_tensor("w2t", [2 * H, H], BF16, kind="ExternalInput")
    wct = nc.dram_tensor("wct", [H, C], BF16, kind="ExternalInput")
    outd = nc.dram_tensor("out", [D0SH, C], F32, kind="ExternalOutput")

    rg = [list(range(NCORES))]
    relu = mybir.ActivationFunctionType.Relu

    with tile.TileContext(nc) as tc:
        with (
            tc.tile_pool(name="const", bufs=1) as constp,
            tc.tile_pool(name="stream", bufs=6) as streamp,
            tc.tile_pool(name="stage", bufs=3) as stagep,
            tc.tile_pool(name="ps1p", bufs=2, space="PSUM") as ps1p,
            tc.tile_pool(name="ps2p", bufs=1, space="PSUM") as ps2p,
            tc.tile_pool(name="psAp", bufs=1, space="PSUM") as psAp,
            tc.tile_pool(name="dram", bufs=1, space="DRAM") as dramp,
        ):
            S_sb = constp.tile([128, NPAIR, 2, F], F8, name="S_sb")
            dfT2_sb = constp.tile([F, CTOT * 128], BF16, name="dfT2_sb")
            d0a_sb = constp.tile([128, CTOT, NCORES, 2 * H], F8, name="d0a_sb")
            w1_sb = constp.tile([128, 2, H], BF16, name="w1_sb")
            w2_sb = constp.tile([128, 2, H], BF16, name="w2_sb")
            wc_sb = constp.tile([H, C], BF16, name="wc_sb")
            h1n = constp.tile([128, CTOT, F], BF16, name="h1n")

            # constant loads.  S_sb gates the stream so it loads first on SP;
            # small consts go via the scalar queue; d0a (needed only for the
            # level-0 partials) is deferred to mid-stream on the scalar queue.
            nc.sync.dma_start(S_sb[:], sfeat.ap())
            nc.scalar.dma_start(dfT2_sb[:], dfT2.ap())
            nc.scalar.dma_start(w1_sb[:], w1t.ap().rearrange("(c p) e -> p c e", p=128))
            nc.scalar.dma_start(w2_sb[:], w2t.ap().rearrange("(c p) e -> p c e", p=128))
            nc.scalar.dma_start(wc_sb[:], wct.ap())

            for _rep in range(repeat):
                # eight [H, 256] level-0 partial accumulators in one 4-bank
                # PSUM tile; the start flag zeroes whole 2KB banks, so only
                # even targets start and only odd targets stop each bank group
                psA = psAp.tile([H, NCORES, 2 * H], F32, tag="psA", name="psA")
                ar_ins, rs_outs = [], []
                for g in range(NAG):
                    ar_ins.append(
                        dramp.tile([GW[g] * F // 128, 128], BF16,
                                   name=f"ar_in{_rep}_{g}")
                    )
                    rs_outs.append(
                        dramp.tile([OWN[g] * F // 128, 128], BF16,
                                   name=f"rs_out{_rep}_{g}")
                    )
                rs2_in = dramp.tile([NCORES * H, 2 * H], BF16, name=f"rs2i{_rep}")
                rs2_out = dramp.tile([H, 2 * H], BF16, name=f"rs2o{_rep}")

                # ---- streamed fp8 DoubleRow matmul: agg1T partial [128, 6144]
                # staged per dst block as feature-major 128-col chunks.  The
                # j-loop emits ONLY stream + staging: all engine queues are
                # in-order, so any chain instruction emitted mid-loop would
                # head-of-line-block later stream work on its queue. ----
                jbase = 0
                for g2 in range(NAG):
                    for jj in range(JBG[g2]):
                        j = jbase + jj
                        ps1 = ps1p.tile([F, JW], F32, tag="ps1")
                        pr = 0
                        for g in range(NG):
                            rt = streamp.tile([128, GP, 2, JW], F8, tag="rt")
                            # alternate HWDGE queues: the stream is paced by
                            # queue residency, not the DMA engines themselves
                            eng = nc.sync if (j * NG + g) % 2 == 0 else nc.scalar
                            eng.dma_start(rt[:], difT1.ap()[j, g])
                            for q in range(GP):
                                nc.tensor.matmul(
                                    ps1[:],
                                    S_sb[:, pr, :, :],
                                    rt[:, q, :, :],
                                    start=(pr == 0),
                                    stop=(pr == NPAIR - 1),
                                    perf_mode=mybir.MatmulPerfMode.DoubleRow,
                                )
                                pr += 1
                        if j == 2:
                            nc.scalar.dma_start(d0a_sb[:], d0a.ap())
                        st = stagep.tile([F, JW], BF16, tag="st")
                        nc.vector.tensor_copy(st[:], ps1[:])
                        nc.scalar.dma_start(
                            ar_ins[g2][jj * 4 * 128 : (jj + 1) * 4 * 128, :]
                            .rearrange("(c p) d -> p c d", p=128),
                            st[:].rearrange("p (c d) -> p c d", c=4),
                        )
                    jbase += JBG[g2]

                # ---- per-group chains: RS -> dense -> level-0 partials ----
                for g2 in range(NAG):
                    oc = OWNC[g2]
                    cbase = sum(OWNC[:g2])
                    nc.gpsimd.collective_compute(
                        "ReduceScatter",
                        mybir.AluOpType.add,
                        replica_groups=rg,
                        ins=[ar_ins[g2].opt()],
                        outs=[rs_outs[g2].opt()],
                    )
                    # RS output rows are (chunk, feat) pairs: readback is
                    # directly agg1^T for this core's own dst rows
                    aggT = stagep.tile([F, oc, 128], BF16, tag=f"aggT{g2}")
                    nc.sync.dma_start(
                        aggT[:],
                        rs_outs[g2][:].rearrange("(c p) d -> p c d", p=128),
                    )
                    ps2 = ps2p.tile([F, OWNC[0] * 128], F32, tag="ps2")
                    aggT2 = aggT[:].rearrange("p c d -> p (c d)")
                    # matmul output must stay within one 2KB PSUM bank
                    for off in range(0, oc * 128, 512):
                        w = min(512, oc * 128 - off)
                        nc.tensor.matmul(
                            ps2[:, off : off + w],
                            w1_sb[:, 0, :],
                            aggT2[:, off : off + w],
                            start=True,
                            stop=False,
                        )
                        nc.tensor.matmul(
                            ps2[:, off : off + w],
                            w1_sb[:, 1, :],
                            dfT2_sb[:, cbase * 128 + off : cbase * 128 + off + w],
                            start=False,
                            stop=True,
                        )
                    h1T = stagep.tile([F, oc * 128], BF16, tag=f"h1T{g2}")
                    nc.scalar.activation(h1T[:], ps2[:, : oc * 128], relu)
                    nc.scalar.dma_start_transpose(
                        h1n[:, cbase : cbase + oc, :], h1T[:]
                    )
                    # own-rows contribution to every core's level-0 sum.
                    # In the last group, sweep kt-banks in order so each
                    # PSUM bank closes early and its a2s copy overlaps the
                    # remaining partial matmuls.
                    if g2 < NAG - 1:
                        for c in range(cbase, cbase + oc):
                            for kt in range(NCORES):
                                nc.tensor.matmul(
                                    psA[:, kt, :],
                                    h1n[:, c, :],
                                    d0a_sb[:, c, kt, :],
                                    start=(c == 0 and kt % 2 == 0),
                                    stop=False,
                                )
                    else:
                        for kt in range(NCORES):
                            for c in range(cbase, cbase + oc):
                                nc.tensor.matmul(
                                    psA[:, kt, :],
                                    h1n[:, c, :],
                                    d0a_sb[:, c, kt, :],
                                    start=False,
                                    stop=(c == CTOT - 1 and kt % 2 == 1),
                                )

                # ---- sum the eight partial stacks in-flight: RS hands each
                # core its own 128 targets' [agg0^T | dst0^T] directly ----
                a2s = stagep.tile([H, NCORES, 2 * H], BF16, tag="a2s")
                for b in range(4):
                    ks = slice(2 * b, 2 * b + 2)
                    nc.vector.tensor_copy(a2s[:, ks, :], psA[:, ks, :])
                    nc.scalar.dma_start(
                        rs2_in[256 * b : 256 * b + 256, :].rearrange(
                            "(k p) e -> p k e", p=128
                        ),
                        a2s[:, ks, :],
                    )
                nc.gpsimd.collective_compute(
                    "ReduceScatter",
                    mybir.AluOpType.add,
                    replica_groups=rg,
                    ins=[rs2_in.opt()],
                    outs=[rs2_out.opt()],
                )
                c0b = stagep.tile([H, 2 * H], BF16, tag="c0b")
                nc.sync.dma_start(c0b[:], rs2_out[:])

                # ---- level-0 dense + classifier + softmax ----
                ps3 = ps2p.tile([H, D0SH], F32, tag="ps34")
                nc.tensor.matmul(
                    ps3[:], w2_sb[:, 0, :], c0b[:, 0:H], start=True, stop=False
                )
                nc.tensor.matmul(
                    ps3[:], w2_sb[:, 1, :], c0b[:, H : 2 * H], start=False, stop=True
                )
                h0T = stagep.tile([H, D0SH], BF16, tag="h0T")
                nc.scalar.activation(h0T[:], ps3[:], relu)
                ps4 = ps2p.tile([D0SH, C], F32, tag="ps34")
                nc.tensor.matmul(ps4[:], h0T[:], wc_sb[:], start=True, stop=True)

                # softmax without max-subtraction: logits here are O(1), so
                # f32 exp cannot overflow and the shift is unnecessary
                esb = stagep.tile([D0SH, C], F32, tag="esb")
                ssum = stagep.tile([D0SH, 1], F32, tag="ssum")
                nc.scalar.activation(
                    esb[:],
                    ps4[:],
                    mybir.ActivationFunctionType.Exp,
                    accum_out=ssum[:],
                )
                rs = stagep.tile([D0SH, 1], F32, tag="rs")
                nc.vector.reciprocal(rs[:], ssum[:])
                osb = stagep.tile([D0SH, C], F32, tag="osb")
                nc.vector.tensor_scalar_mul(osb[:], esb[:], rs[:])
                nc.sync.dma_start(outd.ap(), osb[:])

    nc.compile()
    return nc


def _own_rows(k):
    """Global dst-row index for each of core k's CTOT*128 own rows."""
    rows = []
    base = 0
    for g in range(NAG):
        for c in range(OWNC[g]):
            for p in range(128):
                rows.append(base + OWN[g] * k + 128 * c + p)
        base += GW[g]
    return np.array(rows)


def _prep_in_maps(
    features,
    src_nodes,
    dst_idx_1,
    src_idx_1,
    dif_mat_1,
    dst_idx_0,
    src_idx_0,
    dif_mat_0,
    w1,
    w2,
    w_cls,
):
    f32 = np.float32
    bf16 = ml_dtypes.bfloat16
    f8 = ml_dtypes.float8_e4m3
    features = np.asarray(features, f32)
    dif_mat_1 = np.asarray(dif_mat_1, f32)
    dif_mat_0 = np.asarray(dif_mat_0, f32)
    src_nodes = np.asarray(src_nodes)
    gsrc = src_nodes[np.asarray(src_idx_1)]  # [48000] rows into features
    gdst = src_nodes[np.asarray(dst_idx_1)]  # [6000]

    # dst features, transposed, padded to 6144 rows
    dfT = np.zeros((F, KP), f32)
    dfT[:, :D1] = features[gdst].T

    # level-0 matrix: scatter-added dif0^T (prescaled by S0) | one-hot E
    difT0exp = np.zeros((KP, D0), f32)
    np.add.at(difT0exp, np.asarray(src_idx_0), dif_mat_0.T * S0)
    E = np.zeros((KP, D0), f32)
    E[np.asarray(dst_idx_0), np.arange(D0)] = 1.0

    w1c = np.ascontiguousarray(w1).astype(f32).copy()
    w1c[:F] /= S1  # undo fp8 prescale of dif_mat_1
    w1c = w1c.astype(bf16)
    w2c = np.ascontiguousarray(w2).astype(f32).copy()
    w2c[:H] /= S0  # undo fp8 prescale of dif_mat_0
    w2c = w2c.astype(bf16)
    wcc = np.ascontiguousarray(w_cls).astype(bf16)

    in_maps = []
    for k in range(NCORES):
        sl = slice(k * CH, (k + 1) * CH)
        # fp8 dif shard, prescaled: A8[src 6144, dst 6144]
        A8 = np.zeros((KP, KP), f8)
        A8[:CH, :D1] = (dif_mat_1[:, sl].T * S1).astype(f8)
        # src = (g*GP + q)*256 + i*128 + p ; dst = j*JW + n
        difT1 = np.ascontiguousarray(
            A8.reshape(NG, GP, 2, 128, JB, JW).transpose(4, 0, 3, 1, 2, 5)
        )

        sfeat = np.zeros((KP, F), f32)
        sfeat[:CH] = features[gsrc[sl]]
        sfeat8 = np.ascontiguousarray(
            sfeat.astype(f8).reshape(NPAIR, 2, 128, F).transpose(2, 0, 1, 3)
        )

        rows = _own_rows(k)
        dfT2 = np.ascontiguousarray(dfT[:, rows])

        # level-0 rows for this core's own h1 rows, all 8 target blocks:
        # [p, c, kt, 0:128] = difT0exp[row, kt-block], [.., 128:256] = E[...]
        Dk = difT0exp[rows].reshape(CTOT, 128, NCORES, H)
        Ek = E[rows].reshape(CTOT, 128, NCORES, H)
        d0a = np.concatenate([Dk, Ek], axis=3).astype(f8)  # [c, p, kt, 256]
        d0a = np.ascontiguousarray(d0a.transpose(1, 0, 2, 3))

        in_maps.append(
            {
                "difT1": difT1,
                "sfeat": sfeat8,
                "dfT2": dfT2.astype(bf16),
                "d0a": d0a,
                "w1t": w1c,
                "w2t": w2c,
                "wct": wcc,
            }
        )
    return in_maps


def kernel(**inputs):
    global _nc, LAST
    if _nc is None:
        _nc = _build()
    in_maps = _prep_in_maps(**inputs)
    res = bass_utils.run_bass_kernel_spmd(
        _nc,
        in_maps,
        core_ids=list(range(NCORES)),
        trace=TRACE,
        **TRACE_KW,
    )
    LAST = res
    out = np.concatenate([res.results[k]["out"] for k in range(NCORES)], axis=0)
    return out.astype(np.float32)


# revision 32
# speedup vs baseline: 1.1590x; 1.1590x over previous
"""GraphSage 2-level mean-aggregator GNN on 8 trn2 NeuronCores.

Strategy (memory-bound: dif_mat_1 [6000, 48000] must stream through the chip):

  * Shard the level-1 diffusion matmul over its CONTRACTION dim (48000 src
    selections): core k owns 6000 src columns.  The dif shard streams as
    fp8_e4m3 (prescaled by S1; the 1/S1 rescale is folded into w1's top
    half host-side) -- numerically safe because agg1 is ~0.003 of the
    concatenated dst-feature magnitude.  fp8 halves both HBM traffic and,
    via DoubleRow double-pumping (2 contraction rows per PE pass), the
    TensorEngine time.
  * Each core computes the partial agg1^T = src_feat^T @ dif^T [128, 6144]
    in 12 dst blocks of 512, staged to DRAM as feature-major [dst-chunk,
    feat, 128] chunks in 2 groups of [8, 4] dst blocks.  The chunked
    layout makes the ReduceScatter flat-split land exactly on chunk
    boundaries, so each core's RS output IS its agg1^T slice -- no
    transposes anywhere on the partial path.
  * Per group: ReduceScatter sums the 8 partials; the core runs the
    level-1 dense (relu(concat@w1)) on its 512/256-row slice only.  The
    group-0 chain hides under the stream tail; group 1 is kept tiny so its
    post-stream RS is latency- not size-bound.
  * Level 0 is linear in h1, so h1 is never redistributed: each core
    multiplies its own h1 rows (DMA-transposed to natural layout) into
    every core's level-0 matrix (host-built fp8 [difT0exp*S0 | one-hot-E]
    rows for its slice, all 8 target blocks), giving a [1024, 256] stack
    of per-target partial sums.  A second ReduceScatter sums the stacks
    in-flight and hands each core its own 128 targets' c0 directly.
  * Final dense + softmax per core on its 128 target rows (1/S0 folded
    into w2's top half); host concatenates the eight [128, 40] outputs.
"""

import sys

import ml_dtypes
import numpy as np

sys.path.insert(0, "/opt/trn_rl_repo")

from concourse import bacc, bass_utils, mybir, tile

F32 = mybir.dt.float32
BF16 = mybir.dt.bfloat16
F8 = mybir.dt.float8e4

# Problem dims (hardcoded per spec)
N, F = 100000, 128
N1, D1, S1 = 60000, 6000, 48000
D0, S0 = 1024, 5000
H, C = 128, 40
NCORES = 8
CH = S1 // NCORES   # 6000 src columns per core
KT = 48             # contraction k-tiles of 128 (6144 = padded 6000)
KP = KT * 128       # 6144
NPAIR = KT // 2     # 24 DoubleRow k-pairs
GP = 8              # pairs per stream DMA
NG = NPAIR // GP    # 3 stream DMAs per dst block
JW = 512            # dst-block width for the big matmul
JB = 12             # dst blocks (6144 = padded 6000)
DP = JB * JW        # 6144
JBG = [12]          # single group: fewest collectives (HW latency-bound)
NAG = len(JBG)
GW = [n * JW for n in JBG]          # dst rows per group: 5120, 1024
OWN = [w // NCORES for w in GW]     # own rows per group: 640, 128
OWNC = [o // 128 for o in OWN]      # own 128-chunks per group: 5, 1
CTOT = sum(OWNC)                    # total own chunks: 6
D0SH = D0 // NCORES                 # 128 target rows per core

TRACE = False
TRACE_KW = {}
LAST = None

_nc = None


def _build(repeat=1):
    nc = bacc.Bacc(
        "TRN2",
        target_bir_lowering=False,
        debug=False,
        enable_asserts=False,
        num_devices=NCORES,
    )
    difT1 = nc.dram_tensor("difT1", [JB, NG, 128, GP, 2, JW], F8, kind="ExternalInput")
    sfeat = nc.dram_tensor("sfeat", [128, NPAIR, 2, F], F8, kind="ExternalInput")
    dfT2 = nc.dram_tensor("dfT2", [F, CTOT * 128], BF16, kind="ExternalInput")
    d0a = nc.dram_tensor("d0a", [128, CTOT, NCORES, 2 * H], F8, kind="ExternalInput")
    w1t = nc.dram_tensor("w1t", [2 * F, H], BF16, kind="ExternalInput")
    w2t = nc.dram_tensor("w2t", [2 * H, H], BF16, kind="ExternalInput")
    wct = nc.dram_tensor("wct", [H, C], BF16, kind="ExternalInput")
    outd = nc.dram_tensor("out", [D0SH, C], F32, kind="ExternalOutput")

    rg = [list(range(NCORES))]
    relu = mybir.ActivationFunctionType.Relu

    with tile.TileContext(nc) as tc:
        with (
            tc.tile_pool(name="const", bufs=1) as constp,
            tc.tile_pool(name="stream", bufs=6) as streamp,
            tc.tile_pool(name="stage", bufs=3) as stagep,
            tc.tile_pool(name="ps1p", bufs=1, space="PSUM") as ps1p,
            tc.tile_pool(name="ps2p", bufs=1, space="PSUM") as ps2p,
            tc.tile_pool(name="psAp", bufs=1, space="PSUM") as psAp,
            tc.tile_pool(name="dram", bufs=1, space="DRAM") as dramp,
        ):
            S_sb = constp.tile([128, NPAIR, 2, F], F8, name="S_sb")
            dfT2_sb = constp.tile([F, CTOT * 128], BF16, name="dfT2_sb")
            d0a_sb = constp.tile([128, CTOT, NCORES, 2 * H], F8, name="d0a_sb")
            w1_sb = constp.tile([128, 2, H], BF16, name="w1_sb")
            w2_sb = constp.tile([128, 2, H], BF16, name="w2_sb")
            wc_sb = constp.tile([H, C], BF16, name="wc_sb")
            h1n = constp.tile([128, CTOT, F], BF16, name="h1n")

            # constant loads.  S_sb gates the stream so it loads first on SP;
            # small consts go via the scalar queue; d0a (needed only for the
            # level-0 partials) is deferred to mid-stream on the scalar queue.
            nc.sync.dma_start(S_sb[:], sfeat.ap())
            nc.scalar.dma_start(dfT2_sb[:], dfT2.ap())
            nc.scalar.dma_start(w1_sb[:], w1t.ap().rearrange("(c p) e -> p c e", p=128))
            nc.scalar.dma_start(w2_sb[:], w2t.ap().rearrange("(c p) e -> p c e", p=128))
            nc.scalar.dma_start(wc_sb[:], wct.ap())

            for _rep in range(repeat):
                # eight [H, 256] level-0 partial accumulators in one 4-bank
                # PSUM tile; the start flag zeroes whole 2KB banks, so only
                # even targets start and only odd targets stop each bank group
                psA = psAp.tile([H, NCORES, 2 * H], F32, tag="psA", name="psA")
                ar_ins, rs_outs = [], []
                for g in range(NAG):
                    ar_ins.append(
                        dramp.tile([GW[g] * F // 128, 128], BF16,
                                   name=f"ar_in{_rep}_{g}")
                    )
                    rs_outs.append(
                        dramp.tile([OWN[g] * F // 128, 128], BF16,
                                   name=f"rs_out{_rep}_{g}")
                    )
                rs2_in = dramp.tile([NCORES * H, 2 * H], BF16, name=f"rs2i{_rep}")
                rs2_out = dramp.tile([H, 2 * H], BF16, name=f"rs2o{_rep}")

                # ---- streamed fp8 DoubleRow matmul: agg1T partial [128, 6144]
                # staged per dst block as feature-major 128-col chunks.  The
                # j-loop emits ONLY stream + staging: all engine queues are
                # in-order, so any chain instruction emitted mid-loop would
                # head-of-line-block later stream work on its queue. ----
                jbase = 0
                for g2 in range(NAG):
                    for jj in range(JBG[g2]):
                        j = jbase + jj
                        ps1 = ps1p.tile([F, JW], F32, tag="ps1")
                        pr = 0
                        for g in range(NG):
                            rt = streamp.tile([128, GP, 2, JW], F8, tag="rt")
                            # alternate HWDGE queues: the stream is paced by
                            # queue residency, not the DMA engines themselves
                            eng = nc.sync if (j * NG + g) % 2 == 0 else nc.scalar
                            eng.dma_start(rt[:], difT1.ap()[j, g])
                            for q in range(GP):
                                nc.tensor.matmul(
                                    ps1[:],
                                    S_sb[:, pr, :, :],
                                    rt[:, q, :, :],
                                    start=(pr == 0),
                                    stop=(pr == NPAIR - 1),
                                    perf_mode=mybir.MatmulPerfMode.DoubleRow,
                                )
                                pr += 1
                        if j == 2:
                            nc.scalar.dma_start(d0a_sb[:], d0a.ap())
                        st = stagep.tile([F, JW], BF16, tag="st")
                        nc.vector.tensor_copy(st[:], ps1[:])
                        nc.scalar.dma_start(
                            ar_ins[g2][jj * 4 * 128 : (jj + 1) * 4 * 128, :]
                            .rearrange("(c p) d -> p c d", p=128),
                            st[:].rearrange("p (c d) -> p c d", c=4),
                        )
                    jbase += JBG[g2]

                # ---- per-group chains: RS -> dense -> level-0 partials ----
                for g2 in range(NAG):
                    oc = OWNC[g2]
                    cbase = sum(OWNC[:g2])
                    nc.gpsimd.collective_compute(
                        "ReduceScatter",
                        mybir.AluOpType.add,
                        replica_groups=rg,
                        ins=[ar_ins[g2].opt()],
                        outs=[rs_outs[g2].opt()],
                    )
                    # RS output rows are (chunk, feat) pairs: readback is
                    # directly agg1^T for this core's own dst rows
                    aggT = stagep.tile([F, oc, 128], BF16, tag=f"aggT{g2}")
                    nc.sync.dma_start(
                        aggT[:],
                        rs_outs[g2][:].rearrange("(c p) d -> p c d", p=128),
                    )
                    ps2 = ps2p.tile([F, OWNC[0] * 128], F32, tag="ps2")
                    aggT2 = aggT[:].rearrange("p c d -> p (c d)")
                    # matmul output must stay within one 2KB PSUM bank
                    for off in range(0, oc * 128, 512):
                        w = min(512, oc * 128 - off)
                        nc.tensor.matmul(
                            ps2[:, off : off + w],
                            w1_sb[:, 0, :],
                            aggT2[:, off : off + w],
                            start=True,
                            stop=False,
                        )
                        nc.tensor.matmul(
                            ps2[:, off : off + w],
                            w1_sb[:, 1, :],
                            dfT2_sb[:, cbase * 128 + off : cbase * 128 + off + w],
                            start=False,
                            stop=True,
                        )
                    h1T = stagep.tile([F, oc * 128], BF16, tag=f"h1T{g2}")
                    nc.scalar.activation(h1T[:], ps2[:, : oc * 128], relu)
                    nc.scalar.dma_start_transpose(
                        h1n[:, cbase : cbase + oc, :], h1T[:]
                    )
                    # own-rows contribution to every core's level-0 sum.
                    # In the last group, sweep kt-banks in order so each
                    # PSUM bank closes early and its a2s copy overlaps the
                    # remaining partial matmuls.
                    if g2 < NAG - 1:
                        for c in range(cbase, cbase + oc):
                            for kt in range(NCORES):
                                nc.tensor.matmul(
                                    psA[:, kt, :],
                                    h1n[:, c, :],
                                    d0a_sb[:, c, kt, :],
                                    start=(c == 0 and kt % 2 == 0),
                                    stop=False,
                                )
                    else:
                        for kt in range(NCORES):
                            for c in range(cbase, cbase + oc):
                                nc.tensor.matmul(
                                    psA[:, kt, :],
                                    h1n[:, c, :],
                                    d0a_sb[:, c, kt, :],
                                    start=(cbase == 0 and c == 0 and kt % 2 == 0),
                                    stop=(c == CTOT - 1 and kt % 2 == 1),
                                )

                # ---- sum the eight partial stacks in-flight: RS hands each
                # core its own 128 targets' [agg0^T | dst0^T] directly ----
                a2s = stagep.tile([H, NCORES, 2 * H], BF16, tag="a2s")
                for b in range(4):
                    ks = slice(2 * b, 2 * b + 2)
                    nc.vector.tensor_copy(a2s[:, ks, :], psA[:, ks, :])
                    nc.scalar.dma_start(
                        rs2_in[256 * b : 256 * b + 256, :].rearrange(
                            "(k p) e -> p k e", p=128
                        ),
                        a2s[:, ks, :],
                    )
                nc.gpsimd.collective_compute(
                    "ReduceScatter",
                    mybir.AluOpType.add,
                    replica_groups=rg,
                    ins=[rs2_in.opt()],
                    outs=[rs2_out.opt()],
                )
                c0b = stagep.tile([H, 2 * H], BF16, tag="c0b")
                nc.sync.dma_start(c0b[:], rs2_out[:])

                # ---- level-0 dense + classifier + softmax ----
                ps3 = ps2p.tile([H, D0SH], F32, tag="ps34")
                nc.tensor.matmul(
                    ps3[:], w2_sb[:, 0, :], c0b[:, 0:H], start=True, stop=False
                )
                nc.tensor.matmul(
                    ps3[:], w2_sb[:, 1, :], c0b[:, H : 2 * H], start=False, stop=True
                )
                h0T = stagep.tile([H, D0SH], BF16, tag="h0T")
                nc.scalar.activation(h0T[:], ps3[:], relu)
                ps4 = ps2p.tile([D0SH, C], F32, tag="ps34")
                nc.tensor.matmul(ps4[:], h0T[:], wc_sb[:], start=True, stop=True)

                # softmax without max-subtraction: logits here are O(1), so
                # f32 exp cannot overflow and the shift is unnecessary
                esb = stagep.tile([D0SH, C], F32, tag="esb")
                ssum = stagep.tile([D0SH, 1], F32, tag="ssum")
                nc.scalar.activation(
                    esb[:],
                    ps4[:],
                    mybir.ActivationFunctionType.Exp,
                    accum_out=ssum[:],
                )
                rs = stagep.tile([D0SH, 1], F32, tag="rs")
                nc.vector.reciprocal(rs[:], ssum[:])
                osb = stagep.tile([D0SH, C], F32, tag="osb")
                nc.vector.tensor_scalar_mul(osb[:], esb[:], rs[:])
                nc.sync.dma_start(outd.ap(), osb[:])

    nc.compile()
    return nc


def _own_rows(k):
    """Global dst-row index for each of core k's CTOT*128 own rows."""
    rows = []
    base = 0
    for g in range(NAG):
        for c in range(OWNC[g]):
            for p in range(128):
                rows.append(base + OWN[g] * k + 128 * c + p)
        base += GW[g]
    return np.array(rows)


def _prep_in_maps(
    features,
    src_nodes,
    dst_idx_1,
    src_idx_1,
    dif_mat_1,
    dst_idx_0,
    src_idx_0,
    dif_mat_0,
    w1,
    w2,
    w_cls,
):
    f32 = np.float32
    bf16 = ml_dtypes.bfloat16
    f8 = ml_dtypes.float8_e4m3
    features = np.asarray(features, f32)
    dif_mat_1 = np.asarray(dif_mat_1, f32)
    dif_mat_0 = np.asarray(dif_mat_0, f32)
    src_nodes = np.asarray(src_nodes)
    gsrc = src_nodes[np.asarray(src_idx_1)]  # [48000] rows into features
    gdst = src_nodes[np.asarray(dst_idx_1)]  # [6000]

    # dst features, transposed, padded to 6144 rows
    dfT = np.zeros((F, KP), f32)
    dfT[:, :D1] = features[gdst].T

    # level-0 matrix: scatter-added dif0^T (prescaled by S0) | one-hot E
    difT0exp = np.zeros((KP, D0), f32)
    np.add.at(difT0exp, np.asarray(src_idx_0), dif_mat_0.T * S0)
    E = np.zeros((KP, D0), f32)
    E[np.asarray(dst_idx_0), np.arange(D0)] = 1.0

    w1c = np.ascontiguousarray(w1).astype(f32).copy()
    w1c[:F] /= S1  # undo fp8 prescale of dif_mat_1
    w1c = w1c.astype(bf16)
    w2c = np.ascontiguousarray(w2).astype(f32).copy()
    w2c[:H] /= S0  # undo fp8 prescale of dif_mat_0
    w2c = w2c.astype(bf16)
    wcc = np.ascontiguousarray(w_cls).astype(bf16)

    in_maps = []
    for k in range(NCORES):
        sl = slice(k * CH, (k + 1) * CH)
        # fp8 dif shard, prescaled: A8[src 6144, dst 6144]
        A8 = np.zeros((KP, KP), f8)
        A8[:CH, :D1] = (dif_mat_1[:, sl].T * S1).astype(f8)
        # src = (g*GP + q)*256 + i*128 + p ; dst = j*JW + n
        difT1 = np.ascontiguousarray(
            A8.reshape(NG, GP, 2, 128, JB, JW).transpose(4, 0, 3, 1, 2, 5)
        )

        sfeat = np.zeros((KP, F), f32)
        sfeat[:CH] = features[gsrc[sl]]
        sfeat8 = np.ascontiguousarray(
            sfeat.astype(f8).reshape(NPAIR, 2, 128, F).transpose(2, 0, 1, 3)
        )

        rows = _own_rows(k)
        dfT2 = np.ascontiguousarray(dfT[:, rows])

        # level-0 rows for this core's own h1 rows, all 8 target blocks:
        # [p, c, kt, 0:128] = difT0exp[row, kt-block], [.., 128:256] = E[...]
        Dk = difT0exp[rows].reshape(CTOT, 128, NCORES, H)
        Ek = E[rows].reshape(CTOT, 128, NCORES, H)
        d0a = np.concatenate([Dk, Ek], axis=3).astype(f8)  # [c, p, kt, 256]
        d0a = np.ascontiguousarray(d0a.transpose(1, 0, 2, 3))

        in_maps.append(
            {
                "difT1": difT1,
                "sfeat": sfeat8,
                "dfT2": dfT2.astype(bf16),
                "d0a": d0a,
                "w1t": w1c,
                "w2t": w2c,
                "wct": wcc,
            }
        )
    return in_maps


def kernel(**inputs):
    global _nc, LAST
    if _nc is None:
        _nc = _build()
    in_maps = _prep_in_maps(**inputs)
    res = bass_utils.run_bass_kernel_spmd(
        _nc,
        in_maps,
        core_ids=list(range(NCORES)),
        trace=TRACE,
        **TRACE_KW,
    )
    LAST = res
    out = np.concatenate([res.results[k]["out"] for k in range(NCORES)], axis=0)
    return out.astype(np.float32)


# revision 33
# speedup vs baseline: 1.9251x; 1.6611x over previous
"""GraphSage 2-level mean-aggregator GNN on 8 trn2 NeuronCores.

Strategy (memory-bound: dif_mat_1 [6000, 48000] must stream through the chip):

  * Shard the level-1 diffusion matmul over its CONTRACTION dim (48000 src
    selections): core k owns 6000 src columns.  The dif shard streams as
    fp8_e4m3 (prescaled by S1; the 1/S1 rescale is folded into w1's top
    half host-side) -- numerically safe because agg1 is ~0.003 of the
    concatenated dst-feature magnitude.  fp8 halves both HBM traffic and,
    via DoubleRow double-pumping (2 contraction rows per PE pass), the
    TensorEngine time.
  * Each core computes the partial agg1^T = src_feat^T @ dif^T [128, 6144]
    in 12 dst blocks of 512, staged to DRAM as feature-major [dst-chunk,
    feat, 128] chunks in 2 groups of [8, 4] dst blocks.  The chunked
    layout makes the ReduceScatter flat-split land exactly on chunk
    boundaries, so each core's RS output IS its agg1^T slice -- no
    transposes anywhere on the partial path.
  * Per group: ReduceScatter sums the 8 partials; the core runs the
    level-1 dense (relu(concat@w1)) on its 512/256-row slice only.  The
    group-0 chain hides under the stream tail; group 1 is kept tiny so its
    post-stream RS is latency- not size-bound.
  * Level 0 is linear in h1, so h1 is never redistributed: each core
    multiplies its own h1 rows (DMA-transposed to natural layout) into
    every core's level-0 matrix (host-built fp8 [difT0exp*S0 | one-hot-E]
    rows for its slice, all 8 target blocks), giving a [1024, 256] stack
    of per-target partial sums.  A second ReduceScatter sums the stacks
    in-flight and hands each core its own 128 targets' c0 directly.
  * Final dense + softmax per core on its 128 target rows (1/S0 folded
    into w2's top half); host concatenates the eight [128, 40] outputs.
"""

import sys

import ml_dtypes
import numpy as np

sys.path.insert(0, "/opt/trn_rl_repo")

from concourse import bacc, bass_utils, mybir, tile

F32 = mybir.dt.float32
BF16 = mybir.dt.bfloat16
F8 = mybir.dt.float8e4

# Problem dims (hardcoded per spec)
N, F = 100000, 128
N1, D1, S1 = 60000, 6000, 48000
D0, S0 = 1024, 5000
H, C = 128, 40
NCORES = 8
CH = S1 // NCORES   # 6000 src columns per core
KT = 48             # contraction k-tiles of 128 (6144 = padded 6000)
KP = KT * 128       # 6144
NPAIR = KT // 2     # 24 DoubleRow k-pairs
GP = 8              # pairs per stream DMA
NG = NPAIR // GP    # 3 stream DMAs per dst block
JW = 512            # dst-block width for the big matmul
JB = 12             # dst blocks (6144 = padded 6000)
DP = JB * JW        # 6144
JBG = [12]          # single group: fewest collectives (HW latency-bound)
NAG = len(JBG)
GW = [n * JW for n in JBG]          # dst rows per group: 5120, 1024
OWN = [w // NCORES for w in GW]     # own rows per group: 640, 128
OWNC = [o // 128 for o in OWN]      # own 128-chunks per group: 5, 1
CTOT = sum(OWNC)                    # total own chunks: 6
D0SH = D0 // NCORES                 # 128 target rows per core

TRACE = False
TRACE_KW = {}
LAST = None

_nc = None


def _build(repeat=1):
    nc = bacc.Bacc(
        "TRN2",
        target_bir_lowering=False,
        debug=False,
        enable_asserts=False,
        num_devices=NCORES,
    )
    difT1 = nc.dram_tensor("difT1", [JB, NG, 128, GP, 2, JW], F8, kind="ExternalInput")
    sfeat = nc.dram_tensor("sfeat", [128, NPAIR, 2, F], F8, kind="ExternalInput")
    dfT2 = nc.dram_tensor("dfT2", [F, CTOT * 128], BF16, kind="ExternalInput")
    d0a = nc.dram_tensor("d0a", [128, CTOT, NCORES, 2 * H], F8, kind="ExternalInput")
    w1t = nc.dram_tensor("w1t", [2 * F, H], BF16, kind="ExternalInput")
    w2t = nc.dram_tensor("w2t", [2 * H, H], BF16, kind="ExternalInput")
    wct = nc.dram_tensor("wct", [H, C], BF16, kind="ExternalInput")
    outd = nc.dram_tensor("out", [D0SH, C], F32, kind="ExternalOutput")

    rg = [list(range(NCORES))]
    relu = mybir.ActivationFunctionType.Relu

    with tile.TileContext(nc) as tc:
        with (
            tc.tile_pool(name="const", bufs=1) as constp,
            tc.tile_pool(name="stream", bufs=6) as streamp,
            tc.tile_pool(name="stage", bufs=3) as stagep,
            tc.tile_pool(name="ps1p", bufs=1, space="PSUM") as ps1p,
            tc.tile_pool(name="ps2p", bufs=1, space="PSUM") as ps2p,
            tc.tile_pool(name="psAp", bufs=1, space="PSUM") as psAp,
            tc.tile_pool(name="dram", bufs=1, space="DRAM") as dramp,
        ):
            S_sb = constp.tile([128, NPAIR, 2, F], F8, name="S_sb")
            dfT2_sb = constp.tile([F, CTOT * 128], BF16, name="dfT2_sb")
            d0a_sb = constp.tile([128, CTOT, NCORES, 2 * H], F8, name="d0a_sb")
            w1_sb = constp.tile([128, 2, H], BF16, name="w1_sb")
            w2_sb = constp.tile([128, 2, H], BF16, name="w2_sb")
            wc_sb = constp.tile([H, C], BF16, name="wc_sb")
            h1n = constp.tile([128, CTOT, F], BF16, name="h1n")

            # constant loads.  S_sb gates the stream so it loads first on SP;
            # small consts go via the scalar queue; d0a (needed only for the
            # level-0 partials) is deferred to mid-stream on the scalar queue.
            nc.sync.dma_start(S_sb[:], sfeat.ap())
            nc.scalar.dma_start(dfT2_sb[:], dfT2.ap())
            nc.scalar.dma_start(w1_sb[:], w1t.ap().rearrange("(c p) e -> p c e", p=128))
            nc.scalar.dma_start(w2_sb[:], w2t.ap().rearrange("(c p) e -> p c e", p=128))
            nc.scalar.dma_start(wc_sb[:], wct.ap())

            for _rep in range(repeat):
                # eight [H, 256] level-0 partial accumulators in one 4-bank
                # PSUM tile; the start flag zeroes whole 2KB banks, so only
                # even targets start and only odd targets stop each bank group
                psA = psAp.tile([H, NCORES, 2 * H], F32, tag="psA", name="psA")
                ar_ins, rs_outs = [], []
                for g in range(NAG):
                    ar_ins.append(
                        dramp.tile([GW[g] * F // 128, 128], F8,
                                   name=f"ar_in{_rep}_{g}")
                    )
                    rs_outs.append(
                        dramp.tile([OWN[g] * F // 128, 128], F8,
                                   name=f"rs_out{_rep}_{g}")
                    )
                rs2_in = dramp.tile([NCORES * H, 2 * H], BF16, name=f"rs2i{_rep}")
                rs2_out = dramp.tile([H, 2 * H], BF16, name=f"rs2o{_rep}")

                # ---- streamed fp8 DoubleRow matmul: agg1T partial [128, 6144]
                # staged per dst block as feature-major 128-col chunks.  The
                # j-loop emits ONLY stream + staging: all engine queues are
                # in-order, so any chain instruction emitted mid-loop would
                # head-of-line-block later stream work on its queue. ----
                jbase = 0
                for g2 in range(NAG):
                    for jj in range(JBG[g2]):
                        j = jbase + jj
                        ps1 = ps1p.tile([F, JW], F32, tag="ps1")
                        pr = 0
                        for g in range(NG):
                            rt = streamp.tile([128, GP, 2, JW], F8, tag="rt")
                            # alternate HWDGE queues: the stream is paced by
                            # queue residency, not the DMA engines themselves
                            eng = nc.sync if (j * NG + g) % 2 == 0 else nc.scalar
                            eng.dma_start(rt[:], difT1.ap()[j, g])
                            for q in range(GP):
                                nc.tensor.matmul(
                                    ps1[:],
                                    S_sb[:, pr, :, :],
                                    rt[:, q, :, :],
                                    start=(pr == 0),
                                    stop=(pr == NPAIR - 1),
                                    perf_mode=mybir.MatmulPerfMode.DoubleRow,
                                )
                                pr += 1
                        if j == 2:
                            nc.scalar.dma_start(d0a_sb[:], d0a.ap())
                        st = stagep.tile([F, JW], F8, tag="st")
                        nc.vector.tensor_copy(st[:], ps1[:])
                        nc.scalar.dma_start(
                            ar_ins[g2][jj * 4 * 128 : (jj + 1) * 4 * 128, :]
                            .rearrange("(c p) d -> p c d", p=128),
                            st[:].rearrange("p (c d) -> p c d", c=4),
                        )
                    jbase += JBG[g2]

                # ---- per-group chains: RS -> dense -> level-0 partials ----
                for g2 in range(NAG):
                    oc = OWNC[g2]
                    cbase = sum(OWNC[:g2])
                    nc.gpsimd.collective_compute(
                        "ReduceScatter",
                        mybir.AluOpType.add,
                        replica_groups=rg,
                        ins=[ar_ins[g2].opt()],
                        outs=[rs_outs[g2].opt()],
                    )
                    # RS output rows are (chunk, feat) pairs: readback is
                    # directly agg1^T for this core's own dst rows
                    aggT = stagep.tile([F, oc, 128], F8, tag=f"aggT{g2}")
                    nc.sync.dma_start(
                        aggT[:],
                        rs_outs[g2][:].rearrange("(c p) d -> p c d", p=128),
                    )
                    ps2 = ps2p.tile([F, OWNC[0] * 128], F32, tag="ps2")
                    aggT2 = aggT[:].rearrange("p c d -> p (c d)")
                    # matmul output must stay within one 2KB PSUM bank
                    for off in range(0, oc * 128, 512):
                        w = min(512, oc * 128 - off)
                        nc.tensor.matmul(
                            ps2[:, off : off + w],
                            w1_sb[:, 0, :],
                            aggT2[:, off : off + w],
                            start=True,
                            stop=False,
                        )
                        nc.tensor.matmul(
                            ps2[:, off : off + w],
                            w1_sb[:, 1, :],
                            dfT2_sb[:, cbase * 128 + off : cbase * 128 + off + w],
                            start=False,
                            stop=True,
                        )
                    h1T = stagep.tile([F, oc * 128], BF16, tag=f"h1T{g2}")
                    nc.scalar.activation(h1T[:], ps2[:, : oc * 128], relu)
                    nc.scalar.dma_start_transpose(
                        h1n[:, cbase : cbase + oc, :], h1T[:]
                    )
                    # own-rows contribution to every core's level-0 sum.
                    # In the last group, sweep kt-banks in order so each
                    # PSUM bank closes early and its a2s copy overlaps the
                    # remaining partial matmuls.
                    if g2 < NAG - 1:
                        for c in range(cbase, cbase + oc):
                            for kt in range(NCORES):
                                nc.tensor.matmul(
                                    psA[:, kt, :],
                                    h1n[:, c, :],
                                    d0a_sb[:, c, kt, :],
                                    start=(c == 0 and kt % 2 == 0),
                                    stop=False,
                                )
                    else:
                        for kt in range(NCORES):
                            for c in range(cbase, cbase + oc):
                                nc.tensor.matmul(
                                    psA[:, kt, :],
                                    h1n[:, c, :],
                                    d0a_sb[:, c, kt, :],
                                    start=(cbase == 0 and c == 0 and kt % 2 == 0),
                                    stop=(c == CTOT - 1 and kt % 2 == 1),
                                )

                # ---- sum the eight partial stacks in-flight: RS hands each
                # core its own 128 targets' [agg0^T | dst0^T] directly ----
                a2s = stagep.tile([H, NCORES, 2 * H], BF16, tag="a2s")
                for b in range(4):
                    ks = slice(2 * b, 2 * b + 2)
                    nc.vector.tensor_copy(a2s[:, ks, :], psA[:, ks, :])
                    nc.scalar.dma_start(
                        rs2_in[256 * b : 256 * b + 256, :].rearrange(
                            "(k p) e -> p k e", p=128
                        ),
                        a2s[:, ks, :],
                    )
                nc.gpsimd.collective_compute(
                    "ReduceScatter",
                    mybir.AluOpType.add,
                    replica_groups=rg,
                    ins=[rs2_in.opt()],
                    outs=[rs2_out.opt()],
                )
                c0b = stagep.tile([H, 2 * H], BF16, tag="c0b")
                nc.sync.dma_start(c0b[:], rs2_out[:])

                # ---- level-0 dense + classifier + softmax ----
                ps3 = ps2p.tile([H, D0SH], F32, tag="ps34")
                nc.tensor.matmul(
                    ps3[:], w2_sb[:, 0, :], c0b[:, 0:H], start=True, stop=False
                )
                nc.tensor.matmul(
                    ps3[:], w2_sb[:, 1, :], c0b[:, H : 2 * H], start=False, stop=True
                )
                h0T = stagep.tile([H, D0SH], BF16, tag="h0T")
                nc.scalar.activation(h0T[:], ps3[:], relu)
                ps4 = ps2p.tile([D0SH, C], F32, tag="ps34")
                nc.tensor.matmul(ps4[:], h0T[:], wc_sb[:], start=True, stop=True)

                # softmax without max-subtraction: logits here are O(1), so
                # f32 exp cannot overflow and the shift is unnecessary
                esb = stagep.tile([D0SH, C], F32, tag="esb")
                ssum = stagep.tile([D0SH, 1], F32, tag="ssum")
                nc.scalar.activation(
                    esb[:],
                    ps4[:],
                    mybir.ActivationFunctionType.Exp,
                    accum_out=ssum[:],
                )
                rs = stagep.tile([D0SH, 1], F32, tag="rs")
                nc.vector.reciprocal(rs[:], ssum[:])
                osb = stagep.tile([D0SH, C], F32, tag="osb")
                nc.vector.tensor_scalar_mul(osb[:], esb[:], rs[:])
                nc.sync.dma_start(outd.ap(), osb[:])

    nc.compile()
    return nc


def _own_rows(k):
    """Global dst-row index for each of core k's CTOT*128 own rows."""
    rows = []
    base = 0
    for g in range(NAG):
        for c in range(OWNC[g]):
            for p in range(128):
                rows.append(base + OWN[g] * k + 128 * c + p)
        base += GW[g]
    return np.array(rows)


def _prep_in_maps(
    features,
    src_nodes,
    dst_idx_1,
    src_idx_1,
    dif_mat_1,
    dst_idx_0,
    src_idx_0,
    dif_mat_0,
    w1,
    w2,
    w_cls,
):
    f32 = np.float32
    bf16 = ml_dtypes.bfloat16
    f8 = ml_dtypes.float8_e4m3
    features = np.asarray(features, f32)
    dif_mat_1 = np.asarray(dif_mat_1, f32)
    dif_mat_0 = np.asarray(dif_mat_0, f32)
    src_nodes = np.asarray(src_nodes)
    gsrc = src_nodes[np.asarray(src_idx_1)]  # [48000] rows into features
    gdst = src_nodes[np.asarray(dst_idx_1)]  # [6000]

    # dst features, transposed, padded to 6144 rows
    dfT = np.zeros((F, KP), f32)
    dfT[:, :D1] = features[gdst].T

    # level-0 matrix: scatter-added dif0^T (prescaled by S0) | one-hot E
    difT0exp = np.zeros((KP, D0), f32)
    np.add.at(difT0exp, np.asarray(src_idx_0), dif_mat_0.T * S0)
    E = np.zeros((KP, D0), f32)
    E[np.asarray(dst_idx_0), np.arange(D0)] = 1.0

    w1c = np.ascontiguousarray(w1).astype(f32).copy()
    w1c[:F] /= S1 / 4  # undo fp8 prescale of dif_mat_1
    w1c = w1c.astype(bf16)
    w2c = np.ascontiguousarray(w2).astype(f32).copy()
    w2c[:H] /= S0  # undo fp8 prescale of dif_mat_0
    w2c = w2c.astype(bf16)
    wcc = np.ascontiguousarray(w_cls).astype(bf16)

    in_maps = []
    for k in range(NCORES):
        sl = slice(k * CH, (k + 1) * CH)
        # fp8 dif shard, prescaled: A8[src 6144, dst 6144]
        A8 = np.zeros((KP, KP), f8)
        A8[:CH, :D1] = (dif_mat_1[:, sl].T * (S1 / 4)).astype(f8)
        # src = (g*GP + q)*256 + i*128 + p ; dst = j*JW + n
        difT1 = np.ascontiguousarray(
            A8.reshape(NG, GP, 2, 128, JB, JW).transpose(4, 0, 3, 1, 2, 5)
        )

        sfeat = np.zeros((KP, F), f32)
        sfeat[:CH] = features[gsrc[sl]]
        sfeat8 = np.ascontiguousarray(
            sfeat.astype(f8).reshape(NPAIR, 2, 128, F).transpose(2, 0, 1, 3)
        )

        rows = _own_rows(k)
        dfT2 = np.ascontiguousarray(dfT[:, rows])

        # level-0 rows for this core's own h1 rows, all 8 target blocks:
        # [p, c, kt, 0:128] = difT0exp[row, kt-block], [.., 128:256] = E[...]
        Dk = difT0exp[rows].reshape(CTOT, 128, NCORES, H)
        Ek = E[rows].reshape(CTOT, 128, NCORES, H)
        d0a = np.concatenate([Dk, Ek], axis=3).astype(f8)  # [c, p, kt, 256]
        d0a = np.ascontiguousarray(d0a.transpose(1, 0, 2, 3))

        in_maps.append(
            {
                "difT1": difT1,
                "sfeat": sfeat8,
                "dfT2": dfT2.astype(bf16),
                "d0a": d0a,
                "w1t": w1c,
                "w2t": w2c,
                "wct": wcc,
            }
        )
    return in_maps


def kernel(**inputs):
    global _nc, LAST
    if _nc is None:
        _nc = _build()
    in_maps = _prep_in_maps(**inputs)
    res = bass_utils.run_bass_kernel_spmd(
        _nc,
        in_maps,
        core_ids=list(range(NCORES)),
        trace=TRACE,
        **TRACE_KW,
    )
    LAST = res
    out = np.concatenate([res.results[k]["out"] for k in range(NCORES)], axis=0)
    return out.astype(np.float32)
